# revision 10
# baseline (speedup 1.0000x reference)
"""Bass/Trainium2 kernel for per-chunk fake-quant + linear.

reference semantics (per chunk c):
    q  = clip(round(x/s_c), -128, 127) * s_c
    out[c] = q @ w[c].T          # [B,S,O]

Strategy:
  - Data-parallel over tokens: each of 8 cores gets T = B*S/8 = 8192 tokens
    (all 4 chunks), weights replicated.
  - Host pre-transposes each x shard to [C, D, T] so the contraction dim d
    sits on SBUF partitions (contiguous DMA loads).
  - Integer-domain matmul: qi = clip(rne(x/s), -128, 127) is an integer in
    [-128,127], exactly representable in bf16 -> full-rate bf16 matmuls.
    The scale s is folded into the weights host-side (ws = s*w), split into
    hi/lo bf16 parts for ~f32 accuracy (2 matmuls per k-tile).
  - Rounding via the magic-number trick (+1.5*2^23) = IEEE round-half-even,
    matching jnp.round.
"""

import numpy as np
import ml_dtypes

import concourse.bass as bass
import concourse.tile as tile
import concourse.mybir as mybir
from concourse.bass_utils import run_bass_kernel_spmd
def _split_sync_waits(nc):
    """Hoist excess per-instruction sem waits onto preceding same-engine NOPs.

    This walrus build rejects instructions carrying >2 sync waits ("Too many
    sync wait commands", CoreV2/V3GenImpl setupSyncWait). A NOP on the same
    engine immediately before the instruction blocks the queue identically,
    so semantics are preserved.
    """
    count = 0
    for fn in nc.m.functions:
        for bb in fn.blocks:
            out = []
            for ins in bb.instructions:
                si = ins.sync_info
                waits = list(si.on_wait) if (si and si.on_wait) else []
                maxw = 1
                if len(waits) > maxw:
                    extra, keep = waits[:-maxw], waits[-maxw:]
                    ins.sync_info = mybir.SyncInfo(
                        on_wait=keep, on_update=list(si.on_update or [])
                    )
                    for j in range(0, len(extra), maxw):
                        count += 1
                        nop = mybir.InstNoOp(
                            name=f"ant-waitsplit-{count}", ins=[], outs=[]
                        )
                        nop.engine = ins.engine
                        nop.sync_info = mybir.SyncInfo(
                            on_wait=extra[j : j + maxw], on_update=[]
                        )
                        out.append(nop)
                out.append(ins)
            bb.instructions = out
    return count

C, B, S, D, O = 4, 8, 8192, 256, 256
NCORES = 8
N = B * S            # tokens per chunk (65536)
T = N // NCORES      # tokens per chunk per core (8192)

MAGIC = float(np.float32(1.5 * 2.0**23))  # 12582912.0


def _build_program(scales, t_kern=T, tt=1024):
    """Build the SPMD Bass program (same program on all cores).

    Inputs (per core): xt [C, D, t_kern] f32, wh/wl [C, D, O] bf16.
    Output: out [C, t_kern, O] f32.
    """
    f32 = mybir.dt.float32
    bf16 = mybir.dt.bfloat16
    act = mybir.ActivationFunctionType
    alu = mybir.AluOpType

    assert t_kern % tt == 0 and tt % 128 == 0
    n_tt = t_kern // tt
    n_s4 = tt // 128

    nc = bass.Bass()
    xt = nc.declare_dram_parameter("xt", [C, D, t_kern], f32, isOutput=False)
    wh = nc.declare_dram_parameter("wh", [C, D, O], bf16, isOutput=False)
    wl = nc.declare_dram_parameter("wl", [C, D, O], bf16, isOutput=False)
    out = nc.declare_dram_parameter("out", [C, t_kern, O], f32, isOutput=True)

    with tile.TileContext(nc) as tc:
        with (
            tc.tile_pool(name="wpool", bufs=1) as wpool,
            tc.tile_pool(name="xpool", bufs=3) as xpool,
            tc.tile_pool(name="t1pool", bufs=2) as t1pool,
            tc.tile_pool(name="t2pool", bufs=2) as t2pool,
            tc.tile_pool(name="qpool", bufs=3) as qpool,
            tc.tile_pool(name="opool", bufs=3) as opool,
            tc.tile_pool(name="ppool", bufs=8, space=bass.MemorySpace.PSUM) as ppool,
        ):
            # Per-partition MAGIC bias for the ACT rounding pass.
            magic_bias = wpool.tile([128, 1], f32, tag="magic")
            nc.gpsimd.memset(magic_bias[:], MAGIC)

            # Resident weights: wsT[c][dk] hi/lo, each [128, O] bf16.
            wt = {}
            for c in range(C):
                for dk in range(2):
                    for h, src in ((0, wh), (1, wl)):
                        w_tile = wpool.tile([128, O], bf16, tag=f"w_{c}_{dk}_{h}")
                        nc.sync.dma_start(
                            out=w_tile[:], in_=src[c, dk * 128 : (dk + 1) * 128, :]
                        )
                        wt[c, dk, h] = w_tile

            for c in range(C):
                inv_s = float(np.float32(1.0) / np.float32(scales[c]))
                for it in range(n_tt):
                    # Load x tile: [p=128 (d%128), (dk, t)] from xt[c]
                    x_tile = xpool.tile([128, 2 * tt], f32, tag="x")
                    src = xt[c].rearrange("(dk p) t -> p dk t", dk=2)[
                        :, :, it * tt : (it + 1) * tt
                    ]
                    dst = x_tile[:].rearrange("p (dk t) -> p dk t", dk=2)
                    nc.sync.dma_start(out=dst, in_=src)

                    # t1 = x*inv_s + MAGIC   (rounds to integer+MAGIC, RNE)
                    t1 = t1pool.tile([128, 2 * tt], f32, tag="t1")
                    nc.scalar.activation(
                        t1[:], x_tile[:], act.Identity, bias=magic_bias[:], scale=inv_s
                    )
                    # t2 = min(max(t1, MAGIC-128), MAGIC+127)
                    t2 = t2pool.tile([128, 2 * tt], f32, tag="t2")
                    nc.vector.tensor_scalar(
                        t2[:], t1[:], MAGIC - 128.0, MAGIC + 127.0,
                        alu.max, alu.min,
                    )
                    # qi = t2 - MAGIC  (small integer, exact in bf16)
                    qi = qpool.tile([128, 2 * tt], bf16, tag="qi")
                    nc.vector.tensor_scalar(
                        qi[:], t2[:], MAGIC, None, alu.subtract
                    )

                    # Matmuls: out[t0:t0+128, :] = qi_tile.T @ wsT  (hi+lo)
                    stage = opool.tile([128, n_s4 * O], f32, tag="stage")
                    for s4 in range(n_s4):
                        ps = ppool.tile([128, O], f32, tag="ps")
                        first = True
                        for dk in range(2):
                            lhsT = qi[:, dk * tt + s4 * 128 : dk * tt + s4 * 128 + 128]
                            for h in (0, 1):
                                nc.tensor.matmul(
                                    ps[:], lhsT, wt[c, dk, h][:],
                                    start=first, stop=(dk == 1 and h == 1),
                                )
                                first = False
                        # PSUM -> SBUF staging (alternate engines)
                        if s4 % 2 == 0:
                            nc.scalar.copy(stage[:, s4 * O : (s4 + 1) * O], ps[:])
                        else:
                            nc.vector.tensor_copy(stage[:, s4 * O : (s4 + 1) * O], ps[:])

                    # Store tt tokens: stage [p, (s4, o)] -> out[c, it*tt + s4*128 + p, o]
                    dsto = out[c, it * tt : (it + 1) * tt, :].rearrange(
                        "(s4 p) o -> p s4 o", p=128
                    )
                    nc.scalar.dma_start(
                        out=dsto, in_=stage[:].rearrange("p (s4 o) -> p s4 o", o=O)
                    )
    return nc


def _prep_inputs(x, w, scales, t_kern=T, ncores=NCORES):
    x = np.ascontiguousarray(np.asarray(x, dtype=np.float32)).reshape(C, N, D)
    w = np.asarray(w, dtype=np.float32)
    s = np.asarray(scales, dtype=np.float32).reshape(C, 1, 1)

    ws = s * w                                            # [C, O, D] f32
    wsT = np.ascontiguousarray(ws.transpose(0, 2, 1))     # [C, D, O]
    w_hi = wsT.astype(ml_dtypes.bfloat16)
    w_lo = (wsT - w_hi.astype(np.float32)).astype(ml_dtypes.bfloat16)

    in_maps = []
    for i in range(ncores):
        xs = x[:, i * t_kern : (i + 1) * t_kern, :]       # [C, T, D] view
        xtp = np.ascontiguousarray(xs.transpose(0, 2, 1))  # [C, D, T]
        in_maps.append({"xt": xtp, "wh": w_hi, "wl": w_lo})
    return in_maps


def run(x, w, scales, trace=False, **spmd_kwargs):
    """Compile + run on 8 cores. Returns (out, BassKernelResults)."""
    scales = np.asarray(scales, dtype=np.float32)
    nc = _build_program(scales)
    _split_sync_waits(nc)  # HW-only fixup (CoreSim chokes on raw-BIR NoOps)
    in_maps = _prep_inputs(x, w, scales)
    res = run_bass_kernel_spmd(
        nc, in_maps, core_ids=list(range(NCORES)), trace=trace, **spmd_kwargs
    )
    shards = [r["out"] for r in res.results]              # each [C, T, O]
    out = np.concatenate(shards, axis=1)                  # [C, N, O]
    return np.ascontiguousarray(out).reshape(C, B, S, O), res


def kernel(x, w, scales):
    out, _ = run(x, w, scales, trace=False)
    return out
